# revision 25
# baseline (speedup 1.0000x reference)
"""CoPEGate Trainium2 kernel (v2).

Computes out[b,h,t,s] = sigmoid((Q K^T)[b,h,t,s] / sqrt(D)) * (P P^T)[t,s] / sqrt(D)
for B=2, H=12, T=2048, D=64 (fp32 in/out), distributed over 8 NeuronCores.

Sharding: the 24 (b,h) pairs are split 3-per-core (head-parallel). The
positional bias P P^T / sqrt(D) is computed ON THE HOST (a single
2048x2048x64 GEMM, i.e. input preprocessing of the replicated small
operand per the sharding hint), cast to fp16 in SBUF layout, and shipped
as a replicated input. No cross-device communication.

Why v2 beats v1 (119 us, trace-measured):

1. v1's pacer was ACT: 96 sigmoids of [128,1024] at 997 ns start-to-
   start (= (1024+172)/1.2GHz, exact) = 100.7 us. 2048-wide sigmoids
   run at (2048+172)/1.2 = 1850 ns -> 48 x 1850 = 88.8 us. v1 could
   not go 2048-wide: with the pos bias transiting PSUM, per-tile PSUM
   bank-tenancy (writes + reads) exactly equals the available 2-slot
   time, so wide stripes stall structurally.

2. Shipping pos from the host removes the PE pos matmuls, the 39 us of
   DVE PSUM->SBUF f32->f16 casts (DVE drops to ~59 us of muls, well
   under ACT), and all pos PSUM tenancy -- so gates get all 8 PSUM
   banks as a 2 x [128,2048] ping-pong and ACT streams back-to-back.

3. DMA budget: out 24 MiB + pos 8 MiB + q/k 1.75 MiB = 33.75 MiB vs
   ~358 GB/s/core. pos chunks are size-ramped (0.5/0.5/1/2/4 MiB) and
   issued early so tile j's slice always lands before its muls; the
   ~7 us framework preamble and ramp give the prefetch a head start.

Steady state per row-tile (16 tiles):
  PE : 12 x [128(K),512] fp16 matmul chunks (~4.3 us incl LDWEIGHTS)
  ACT: 3 x [128,2048] sigmoid PSUM->SBUF f16 (1850 ns each; pacer)
  DVE: 3 x [128,2048] fp16 tensor_mul (1226 ns each)
  DMA: 3 x 512 KiB output stripes (+ pos trickle)
Precision: q/k fp16 (f32 psum accumulate), pos f32 host GEMM -> fp16,
fp16 out upcast on host; rel err ~5e-4 vs the 2e-2 gate.
"""

import math
import os
import sys

import numpy as np

sys.path.insert(0, "/opt/trn_rl_repo")

B, H, T, D = 2, 12, 2048, 64
N_CORES = 8
HPC = (B * H) // N_CORES  # heads per core
PT = 128  # output row-tile height (SBUF/PSUM partitions)
NT = T // PT  # row tiles
NCHUNK = 512  # matmul moving-operand free dim (one PSUM bank of fp32)
NCH = T // NCHUNK
INV_SQRT_D = 1.0 / math.sqrt(D)
NSHIP = 8  # pos tiles shipped from host; the rest are computed on-device

# pos-prefetch pacing: pos tile jt's DMA is issued right after the gate
# at (it, h) is produced (a data dependency the scheduler can't hoist).
# ~1 fetch per 2.4 sigmoids ~= 115 GB/s, matching the DMA capacity left
# over by the output stream so neither backlogs. All fetches run 2.5+
# row-tiles ahead of the muls that consume them. (pos0 is gated on rk,
# pos1 on the warmup gate.)
POS_SLOT = {
    (0, 0): 1, (0, 1): 2, (0, 2): 3,
    (1, 1): 4, (1, 2): 5,
    (2, 2): 6,
    (3, 1): 7,
}

_NC_CACHE = {}


def _build_nc():
    import concourse.bass as bass
    from concourse import bacc, mybir, tile

    f32 = mybir.dt.float32
    f16 = mybir.dt.float16
    Sigmoid = mybir.ActivationFunctionType.Sigmoid

    nc = bacc.Bacc("TRN2", target_bir_lowering=False)

    # Host-packed operands:
    #   QZ[h] = q_h^T [64, 2048]; the other 64 rows of each [128, T]
    #   stationary tile are memset to 0 on-device (zero rows contribute
    #   exactly 0 to the K=128 contraction, which runs the PE at 2.4GHz
    #   vs 1.2 for K=64).
    #   RHS[0] = [k0;k1], RHS[1] = [k2;k2] (moving tiles, rows = K).
    #   POS[p, it*T + c] = pos_bias[it*128 + p, c] * inv_sqrt_d (fp16) --
    #   i.e. already in SBUF [partition, tile-major free] layout.
    QZ = nc.dram_tensor("QZ", [HPC, D, T], f16, kind="ExternalInput")
    RHS = nc.dram_tensor("RHS", [2, 2 * D, T], f16, kind="ExternalInput")
    POS = nc.dram_tensor("POS", [PT, NSHIP * T], f16, kind="ExternalInput")
    QP = nc.dram_tensor("QP", [D, T], f16, kind="ExternalInput")
    out = nc.dram_tensor("out", [HPC, T, T], f16, kind="ExternalOutput")

    with tile.TileContext(nc) as tc:
        with tc.tile_pool(name="ins", bufs=1) as ins_pool, \
             tc.tile_pool(name="gate", bufs=6) as gate_pool, \
             tc.tile_pool(name="outs", bufs=12) as outs_pool:

            qz0 = ins_pool.tile([2 * D, T], f16, tag="qz0")
            qz1 = ins_pool.tile([2 * D, T], f16, tag="qz1")
            qz2 = ins_pool.tile([2 * D, T], f16, tag="qz2")
            rk = ins_pool.tile([2 * D, T], f16, tag="rk")
            rp = ins_pool.tile([2 * D, T], f16, tag="rp")
            qp = ins_pool.tile([2 * D, T], f16, tag="qp")
            pos = ins_pool.tile([PT, NT * T], f16, tag="pos")

            # Zero halves: qz0=[q0;0], qz1=[0;q1], qz2=[q2;0].
            # GPSIMD + DVE are idle through the ramp; keep zeros off the
            # DMA wire.
            nc.gpsimd.memset(qz0[D : 2 * D, :], 0.0)
            nc.vector.memset(qz1[0:D, :], 0.0)
            nc.gpsimd.memset(qz2[D : 2 * D, :], 0.0)
            nc.vector.memset(qp[0:D, :], 0.0)

            # Input DMAs in ACT-first-use order: everything the first
            # three sigmoid stripes need goes ahead of the pos bulk
            # (pos is only needed by the muls, which trail by >=2 us
            # behind a 4-deep gate pool). rk is split into 4 chunks so
            # tile-0 matmuls start as soon as the first cols land.
            # One DMA per input tile: the HWDGE queue drains DMAs
            # FIFO with ~0.5-0.6 us of per-transfer latency, so a
            # chunked rk (4 DMAs) pushed rp/qz2 completion past 17 us
            # and stalled the first h1/h2 sigmoids. Order = first use.
            # Ramp-critical first: tile 0 needs only the first column
            # block (16 KiB) of each stationary q operand, so those ship
            # separately ahead of the 240 KiB remainders. rp (moving,
            # 512 KiB, needed whole by the third sigmoid's matmuls) is
            # the long pole; everything after it has >=2 tiles of slack.
            nc.sync.dma_start(out=qz0[0:D, 0:PT], in_=QZ[0][:, 0:PT])
            nc.sync.dma_start(out=rk[:, 0 : T // 2], in_=RHS[0][:, 0 : T // 2])
            nc.sync.dma_start(out=rk[:, T // 2 : T], in_=RHS[0][:, T // 2 : T])
            nc.sync.dma_start(out=qz1[D : 2 * D, 0:PT], in_=QZ[1][:, 0:PT])
            nc.sync.dma_start(out=rp, in_=RHS[1][:, :])
            nc.sync.dma_start(out=qz2[0:D, 0:PT], in_=QZ[2][:, 0:PT])
            nc.sync.dma_start(out=qz0[0:D, PT:T], in_=QZ[0][:, PT:T])
            nc.sync.dma_start(out=qz1[D : 2 * D, PT:T], in_=QZ[1][:, PT:T])
            nc.sync.dma_start(out=qz2[0:D, PT:T], in_=QZ[2][:, PT:T])
            nc.sync.dma_start(out=qp[D : 2 * D, :], in_=QP[:, :])
            # pos rides the SWDGE (gpsimd) descriptor path: its packets
            # interleave with HWDGE output DMAs at the SDMA engines
            # instead of queueing FIFO behind/ahead of them in the same
            # HW ring -- a 4 MiB pos bulk on the sync ring was measured
            # to delay an output DMA by ~20 us and stall ACT.
            #
            # Pacing: an ungated SWDGE stream takes ~50% of DMA
            # bandwidth, which starved the ramp-critical rp/qz2 inputs
            # (512 KiB took 7-12 us) and stalled the early sigmoids.
            # Program order does NOT pace DMAs (the tile scheduler
            # reorders), so each pos-tile DMA is gated by a real data
            # dependency: a 1-element gpsimd copy writes into the first
            # bytes of that pos slice (WAW -> the DMA must follow), and
            # the copy's SOURCE is pipeline state that becomes available
            # ~3 row-tiles before the slice's muls need it. pos tile 0
            # is gated on the last rk chunk (all ramp inputs beat it);
            # tiles 1.. are issued inside the loop off the gate tiles.
            def pos_fetch(jt, anchor):
                # anchor: a [1, 8] slice of data the DMA must wait for.
                nc.gpsimd.tensor_copy(
                    pos[0:1, jt * T : jt * T + 8], anchor
                )
                nc.gpsimd.dma_start(
                    out=pos[:, jt * T : (jt + 1) * T],
                    in_=POS[:, jt * T : (jt + 1) * T],
                )

            pos_fetch(0, rp[0:1, 0:8])

            lhs_t = {0: qz0, 1: qz1, 2: qz2}
            rhs_t = {0: rk, 1: rk, 2: rp}

            def emit_head(ps, it, h, last=False):
                lhsT = lhs_t[h][:, bass.ts(it, PT)]
                gate = gate_pool.tile([PT, T], f16, tag="gate")
                o = outs_pool.tile([PT, T], f16, tag="o")
                pslice = pos[:, it * T : (it + 1) * T]
                for j in range(NCH):
                    jsl = bass.ts(j, NCHUNK)
                    nc.tensor.matmul(
                        ps[:, jsl], lhsT, rhs_t[h][:, jsl],
                        start=True, stop=True,
                    )
                if last:
                    # Tail trim: half-wide sigmoids; the final half's
                    # mul+DMA are split again so the very last chain is
                    # (1024-sig + 512-mul + 128 KiB DMA + receipt).
                    for j in range(2):
                        jsl = bass.ts(j, T // 2)
                        nc.scalar.activation(
                            gate[:, jsl], ps[:, jsl], Sigmoid,
                            scale=INV_SQRT_D,
                        )
                        if j == 0:
                            nc.vector.tensor_mul(
                                o[:, jsl], gate[:, jsl], pslice[:, jsl]
                            )
                            nc.sync.dma_start(
                                out=out[h, bass.ts(it, PT), jsl],
                                in_=o[:, jsl],
                            )
                        else:
                            for q in (2, 3):
                                qsl = bass.ts(q, T // 4)
                                nc.vector.tensor_mul(
                                    o[:, qsl], gate[:, qsl], pslice[:, qsl]
                                )
                                nc.sync.dma_start(
                                    out=out[h, bass.ts(it, PT), qsl],
                                    in_=o[:, qsl],
                                )
                else:
                    nc.scalar.activation(gate, ps, Sigmoid, scale=INV_SQRT_D)
                    nc.vector.tensor_mul(o, gate, pslice)
                    nc.sync.dma_start(out=out[h, bass.ts(it, PT), :], in_=o)
                return gate

            with tc.tile_pool(name="ps", bufs=2, space="PSUM") as ps_pool:
                for it in range(NT):
                    for h in range(HPC):
                        last = it == NT - 1 and h == HPC - 1
                        ps = ps_pool.tile([PT, T], f32, tag="ps")
                        gate = emit_head(ps, it, h, last=last)
                        jt = POS_SLOT.get((it, h))
                        if jt is not None:
                            pos_fetch(jt, gate[0:1, 0:8])
                        # Device-side pos stripes (tiles NSHIP..15), one
                        # per two row-tiles: 4 matmuls [0;p]x[k2;p] into
                        # a full PSUM stripe + a DVE f32->f16 cast into
                        # the pos arena. Fits the 2-slot PSUM ping-pong
                        # slack (one extra 3.1 us tenancy per 11.1 us of
                        # per-2-tile slot time) and cuts 4 MiB off the
                        # DMA wire, keeping slow-DMA runs ACT-bound.
                        if h == 2 and 4 <= it < 4 + (NT - NSHIP):
                            jp = it + NSHIP - 4
                            pp = ps_pool.tile([PT, T], f32, tag="ps")
                            for j in range(NCH):
                                jsl = bass.ts(j, NCHUNK)
                                nc.tensor.matmul(
                                    pp[:, jsl],
                                    qp[:, bass.ts(jp, PT)],
                                    rp[:, jsl],
                                    start=True,
                                    stop=True,
                                )
                            nc.vector.tensor_copy(
                                pos[:, jp * T : (jp + 1) * T], pp
                            )

    nc.finalize()
    return nc


def _get_nc():
    if "nc" not in _NC_CACHE:
        _NC_CACHE["nc"] = _build_nc()
    return _NC_CACHE["nc"]


def kernel(query, key, pos_embed_weight):
    query = np.asarray(query, dtype=np.float32)
    key = np.asarray(key, dtype=np.float32)
    pos_embed_weight = np.asarray(pos_embed_weight, dtype=np.float32)

    q = query.reshape(B * H, T, D)
    k = key.reshape(B * H, T, D)

    # Replicated positional bias: the first NSHIP row-tiles are computed
    # on host (small GEMM over the replicated operand) in f32, pre-scaled,
    # cast once to fp16 in the exact SBUF [partition, tile-major] layout
    # the kernel reads. The remaining tiles are computed on-device from
    # p~ = P^T * D**-0.25 (shipped once), trading DVE slack for 4 MiB of
    # DMA traffic.
    p = pos_embed_weight[:T]
    pos_bias = (p[: NSHIP * PT] @ p.T) * np.float32(INV_SQRT_D)
    posh = (
        pos_bias.astype(np.float16)
        .reshape(NSHIP, PT, T)
        .transpose(1, 0, 2)
        .reshape(PT, NSHIP * T)
    )
    posh = np.ascontiguousarray(posh)
    pt = np.ascontiguousarray(p.T * np.float32(D**-0.25)).astype(np.float16)

    in_maps = []
    for c in range(N_CORES):
        hs = [c * HPC + i for i in range(HPC)]
        qz = np.empty((HPC, D, T), dtype=np.float16)
        for i, h in enumerate(hs):
            qz[i] = q[h].T
        kT = [k[h].T.astype(np.float16) for h in hs]
        rhs = np.empty((2, 2 * D, T), dtype=np.float16)
        rhs[0, :D] = kT[0]
        rhs[0, D:] = kT[1]
        rhs[1, :D] = kT[2]
        rhs[1, D:] = pt
        in_maps.append({"QZ": qz, "RHS": rhs, "POS": posh, "QP": pt})

    from concourse.bass_utils import run_bass_kernel_spmd

    nc = _get_nc()
    try:
        res = run_bass_kernel_spmd(
            nc,
            in_maps,
            core_ids=list(range(N_CORES)),
            trace=bool(os.environ.get("KERNEL_TRACE")),
        )
    except Exception:
        # One retry for transient runtime/compile hiccups.
        res = run_bass_kernel_spmd(
            nc, in_maps, core_ids=list(range(N_CORES)), trace=False
        )
    kernel.last_results = res

    full = np.empty((B * H, T, T), dtype=np.float32)
    for c in range(N_CORES):
        full[c * HPC : (c + 1) * HPC] = res.results[c]["out"]
    return full.reshape(B, H, T, T)


kernel.last_results = None


# revision 26
# speedup vs baseline: 1.0952x; 1.0952x over previous
"""CoPEGate Trainium2 kernel (v2).

Computes out[b,h,t,s] = sigmoid((Q K^T)[b,h,t,s] / sqrt(D)) * (P P^T)[t,s] / sqrt(D)
for B=2, H=12, T=2048, D=64 (fp32 in/out), distributed over 8 NeuronCores.

Sharding: the 24 (b,h) pairs are split 3-per-core (head-parallel). The
positional bias P P^T / sqrt(D) is computed ON THE HOST (a single
2048x2048x64 GEMM, i.e. input preprocessing of the replicated small
operand per the sharding hint), cast to fp16 in SBUF layout, and shipped
as a replicated input. No cross-device communication.

Why v2 beats v1 (119 us, trace-measured):

1. v1's pacer was ACT: 96 sigmoids of [128,1024] at 997 ns start-to-
   start (= (1024+172)/1.2GHz, exact) = 100.7 us. 2048-wide sigmoids
   run at (2048+172)/1.2 = 1850 ns -> 48 x 1850 = 88.8 us. v1 could
   not go 2048-wide: with the pos bias transiting PSUM, per-tile PSUM
   bank-tenancy (writes + reads) exactly equals the available 2-slot
   time, so wide stripes stall structurally.

2. Shipping pos from the host removes the PE pos matmuls, the 39 us of
   DVE PSUM->SBUF f32->f16 casts (DVE drops to ~59 us of muls, well
   under ACT), and all pos PSUM tenancy -- so gates get all 8 PSUM
   banks as a 2 x [128,2048] ping-pong and ACT streams back-to-back.

3. DMA budget: out 24 MiB + pos 8 MiB + q/k 1.75 MiB = 33.75 MiB vs
   ~358 GB/s/core. pos chunks are size-ramped (0.5/0.5/1/2/4 MiB) and
   issued early so tile j's slice always lands before its muls; the
   ~7 us framework preamble and ramp give the prefetch a head start.

Steady state per row-tile (16 tiles):
  PE : 12 x [128(K),512] fp16 matmul chunks (~4.3 us incl LDWEIGHTS)
  ACT: 3 x [128,2048] sigmoid PSUM->SBUF f16 (1850 ns each; pacer)
  DVE: 3 x [128,2048] fp16 tensor_mul (1226 ns each)
  DMA: 3 x 512 KiB output stripes (+ pos trickle)
Precision: q/k fp16 (f32 psum accumulate), pos f32 host GEMM -> fp16,
fp16 out upcast on host; rel err ~5e-4 vs the 2e-2 gate.
"""

import math
import os
import sys

import numpy as np

sys.path.insert(0, "/opt/trn_rl_repo")

B, H, T, D = 2, 12, 2048, 64
N_CORES = 8
HPC = (B * H) // N_CORES  # heads per core
PT = 128  # output row-tile height (SBUF/PSUM partitions)
NT = T // PT  # row tiles
NCHUNK = 512  # matmul moving-operand free dim (one PSUM bank of fp32)
NCH = T // NCHUNK
INV_SQRT_D = 1.0 / math.sqrt(D)

# pos-prefetch pacing: pos tile jt's DMA is issued right after the gate
# at (it, h) is produced (a data dependency the scheduler can't hoist).
# ~1 fetch per 2.4 sigmoids ~= 115 GB/s, matching the DMA capacity left
# over by the output stream so neither backlogs. All fetches run 2.5+
# row-tiles ahead of the muls that consume them. (pos0 is gated on rk,
# pos1 on the warmup gate.)
POS_SLOT = {
    (0, 0): 1, (0, 1): 2, (0, 2): 3,
    (1, 1): 4, (1, 2): 5,
    (2, 2): 6,
    (3, 1): 7, (3, 2): 8,
    (4, 2): 9,
    (5, 1): 10, (5, 2): 11,
    (6, 2): 12,
    (7, 1): 13, (7, 2): 14,
    (8, 2): 15,
}

_NC_CACHE = {}


def _build_nc():
    import concourse.bass as bass
    from concourse import bacc, mybir, tile

    f32 = mybir.dt.float32
    f16 = mybir.dt.float16
    Sigmoid = mybir.ActivationFunctionType.Sigmoid

    nc = bacc.Bacc("TRN2", target_bir_lowering=False)

    # Host-packed operands:
    #   QZ[h] = q_h^T [64, 2048]; the other 64 rows of each [128, T]
    #   stationary tile are memset to 0 on-device (zero rows contribute
    #   exactly 0 to the K=128 contraction, which runs the PE at 2.4GHz
    #   vs 1.2 for K=64).
    #   RHS[0] = [k0;k1], RHS[1] = [k2;k2] (moving tiles, rows = K).
    #   POS[p, it*T + c] = pos_bias[it*128 + p, c] * inv_sqrt_d (fp16) --
    #   i.e. already in SBUF [partition, tile-major free] layout.
    QZ = nc.dram_tensor("QZ", [HPC, D, T], f16, kind="ExternalInput")
    RHS = nc.dram_tensor("RHS", [2, 2 * D, T], f16, kind="ExternalInput")
    POS = nc.dram_tensor("POS", [PT, NT * T], f16, kind="ExternalInput")
    out = nc.dram_tensor("out", [HPC, T, T], f16, kind="ExternalOutput")

    with tile.TileContext(nc) as tc:
        with tc.tile_pool(name="ins", bufs=1) as ins_pool, \
             tc.tile_pool(name="gate", bufs=6) as gate_pool, \
             tc.tile_pool(name="outs", bufs=12) as outs_pool:

            qz0 = ins_pool.tile([2 * D, T], f16, tag="qz0")
            qz1 = ins_pool.tile([2 * D, T], f16, tag="qz1")
            qz2 = ins_pool.tile([2 * D, T], f16, tag="qz2")
            rk = ins_pool.tile([2 * D, T], f16, tag="rk")
            rp = ins_pool.tile([2 * D, T], f16, tag="rp")
            pos = ins_pool.tile([PT, NT * T], f16, tag="pos")

            # Zero halves: qz0=[q0;0], qz1=[0;q1], qz2=[q2;0].
            # GPSIMD + DVE are idle through the ramp; keep zeros off the
            # DMA wire.
            nc.gpsimd.memset(qz0[D : 2 * D, :], 0.0)
            nc.vector.memset(qz1[0:D, :], 0.0)
            nc.gpsimd.memset(qz2[D : 2 * D, :], 0.0)

            # Input DMAs in ACT-first-use order: everything the first
            # three sigmoid stripes need goes ahead of the pos bulk
            # (pos is only needed by the muls, which trail by >=2 us
            # behind a 4-deep gate pool). rk is split into 4 chunks so
            # tile-0 matmuls start as soon as the first cols land.
            # One DMA per input tile: the HWDGE queue drains DMAs
            # FIFO with ~0.5-0.6 us of per-transfer latency, so a
            # chunked rk (4 DMAs) pushed rp/qz2 completion past 17 us
            # and stalled the first h1/h2 sigmoids. Order = first use.
            # Ramp-critical first: tile 0 needs only the first column
            # block (16 KiB) of each stationary q operand, so those ship
            # separately ahead of the 240 KiB remainders. rp (moving,
            # 512 KiB, needed whole by the third sigmoid's matmuls) is
            # the long pole; everything after it has >=2 tiles of slack.
            nc.sync.dma_start(out=qz0[0:D, 0:PT], in_=QZ[0][:, 0:PT])
            nc.sync.dma_start(out=rk[:, 0 : T // 2], in_=RHS[0][:, 0 : T // 2])
            nc.sync.dma_start(out=rk[:, T // 2 : T], in_=RHS[0][:, T // 2 : T])
            nc.sync.dma_start(out=qz1[D : 2 * D, 0:PT], in_=QZ[1][:, 0:PT])
            nc.sync.dma_start(out=rp, in_=RHS[1][:, :])
            nc.sync.dma_start(out=qz2[0:D, 0:PT], in_=QZ[2][:, 0:PT])
            nc.sync.dma_start(out=qz0[0:D, PT:T], in_=QZ[0][:, PT:T])
            nc.sync.dma_start(out=qz1[D : 2 * D, PT:T], in_=QZ[1][:, PT:T])
            nc.sync.dma_start(out=qz2[0:D, PT:T], in_=QZ[2][:, PT:T])
            # pos rides the SWDGE (gpsimd) descriptor path: its packets
            # interleave with HWDGE output DMAs at the SDMA engines
            # instead of queueing FIFO behind/ahead of them in the same
            # HW ring -- a 4 MiB pos bulk on the sync ring was measured
            # to delay an output DMA by ~20 us and stall ACT.
            #
            # Pacing: an ungated SWDGE stream takes ~50% of DMA
            # bandwidth, which starved the ramp-critical rp/qz2 inputs
            # (512 KiB took 7-12 us) and stalled the early sigmoids.
            # Program order does NOT pace DMAs (the tile scheduler
            # reorders), so each pos-tile DMA is gated by a real data
            # dependency: a 1-element gpsimd copy writes into the first
            # bytes of that pos slice (WAW -> the DMA must follow), and
            # the copy's SOURCE is pipeline state that becomes available
            # ~3 row-tiles before the slice's muls need it. pos tile 0
            # is gated on the last rk chunk (all ramp inputs beat it);
            # tiles 1.. are issued inside the loop off the gate tiles.
            def pos_fetch(jt, anchor):
                # anchor: a [1, 8] slice of data the DMA must wait for.
                nc.gpsimd.tensor_copy(
                    pos[0:1, jt * T : jt * T + 8], anchor
                )
                nc.sync.dma_start(
                    out=pos[:, jt * T : (jt + 1) * T],
                    in_=POS[:, jt * T : (jt + 1) * T],
                )

            pos_fetch(0, rp[0:1, 0:8])

            lhs_t = {0: qz0, 1: qz1, 2: qz2}
            rhs_t = {0: rk, 1: rk, 2: rp}

            def emit_head(ps, it, h, last=False):
                lhsT = lhs_t[h][:, bass.ts(it, PT)]
                gate = gate_pool.tile([PT, T], f16, tag="gate")
                o = outs_pool.tile([PT, T], f16, tag="o")
                pslice = pos[:, it * T : (it + 1) * T]
                for j in range(NCH):
                    jsl = bass.ts(j, NCHUNK)
                    nc.tensor.matmul(
                        ps[:, jsl], lhsT, rhs_t[h][:, jsl],
                        start=True, stop=True,
                    )
                if last:
                    # Tail trim: half-wide sigmoids; the final half's
                    # mul+DMA are split again so the very last chain is
                    # (1024-sig + 512-mul + 128 KiB DMA + receipt).
                    for j in range(2):
                        jsl = bass.ts(j, T // 2)
                        nc.scalar.activation(
                            gate[:, jsl], ps[:, jsl], Sigmoid,
                            scale=INV_SQRT_D,
                        )
                        if j == 0:
                            nc.vector.tensor_mul(
                                o[:, jsl], gate[:, jsl], pslice[:, jsl]
                            )
                            nc.sync.dma_start(
                                out=out[h, bass.ts(it, PT), jsl],
                                in_=o[:, jsl],
                            )
                        else:
                            for q in (2, 3):
                                qsl = bass.ts(q, T // 4)
                                nc.vector.tensor_mul(
                                    o[:, qsl], gate[:, qsl], pslice[:, qsl]
                                )
                                nc.sync.dma_start(
                                    out=out[h, bass.ts(it, PT), qsl],
                                    in_=o[:, qsl],
                                )
                else:
                    nc.scalar.activation(gate, ps, Sigmoid, scale=INV_SQRT_D)
                    nc.vector.tensor_mul(o, gate, pslice)
                    nc.sync.dma_start(out=out[h, bass.ts(it, PT), :], in_=o)
                return gate

            with tc.tile_pool(name="ps", bufs=2, space="PSUM") as ps_pool:
                for it in range(NT):
                    for h in range(HPC):
                        last = it == NT - 1 and h == HPC - 1
                        ps = ps_pool.tile([PT, T], f32, tag="ps")
                        gate = emit_head(ps, it, h, last=last)
                        jt = POS_SLOT.get((it, h))
                        if jt is not None:
                            pos_fetch(jt, gate[0:1, 0:8])

    nc.finalize()
    return nc


def _get_nc():
    if "nc" not in _NC_CACHE:
        _NC_CACHE["nc"] = _build_nc()
    return _NC_CACHE["nc"]


def kernel(query, key, pos_embed_weight):
    query = np.asarray(query, dtype=np.float32)
    key = np.asarray(key, dtype=np.float32)
    pos_embed_weight = np.asarray(pos_embed_weight, dtype=np.float32)

    q = query.reshape(B * H, T, D)
    k = key.reshape(B * H, T, D)

    # Replicated positional bias: the first NSHIP row-tiles are computed
    # on host (small GEMM over the replicated operand) in f32, pre-scaled,
    # cast once to fp16 in the exact SBUF [partition, tile-major] layout
    # the kernel reads. The remaining tiles are computed on-device from
    # p~ = P^T * D**-0.25 (shipped once), trading DVE slack for 4 MiB of
    # DMA traffic.
    p = pos_embed_weight[:T]
    pos_bias = (p @ p.T) * np.float32(INV_SQRT_D)
    posh = (
        pos_bias.astype(np.float16)
        .reshape(NT, PT, T)
        .transpose(1, 0, 2)
        .reshape(PT, NT * T)
    )
    posh = np.ascontiguousarray(posh)

    in_maps = []
    for c in range(N_CORES):
        hs = [c * HPC + i for i in range(HPC)]
        qz = np.empty((HPC, D, T), dtype=np.float16)
        for i, h in enumerate(hs):
            qz[i] = q[h].T
        kT = [k[h].T.astype(np.float16) for h in hs]
        rhs = np.empty((2, 2 * D, T), dtype=np.float16)
        rhs[0, :D] = kT[0]
        rhs[0, D:] = kT[1]
        rhs[1, :D] = kT[2]
        rhs[1, D:] = kT[2]
        in_maps.append({"QZ": qz, "RHS": rhs, "POS": posh})

    from concourse.bass_utils import run_bass_kernel_spmd

    nc = _get_nc()
    try:
        res = run_bass_kernel_spmd(
            nc,
            in_maps,
            core_ids=list(range(N_CORES)),
            trace=bool(os.environ.get("KERNEL_TRACE")),
        )
    except Exception:
        # One retry for transient runtime/compile hiccups.
        res = run_bass_kernel_spmd(
            nc, in_maps, core_ids=list(range(N_CORES)), trace=False
        )
    kernel.last_results = res

    full = np.empty((B * H, T, T), dtype=np.float32)
    for c in range(N_CORES):
        full[c * HPC : (c + 1) * HPC] = res.results[c]["out"]
    return full.reshape(B, H, T, T)


kernel.last_results = None
